# revision 1
# baseline (speedup 1.0000x reference)
"""Trainium2 Bass kernel for nn_AttentionModule (sparse_attention).

Computation (reference):
  q = tanh(einsum('hde,be->hbd', Query, x))          H=8 D=256 E=1536
  k = tanh(einsum('hdf,blf->hbld', Key, bank))       B=64 L=256 F=768
  s = einsum('hbld,hbd->hbl', k, q)  masked softmax over l
  out = LeakyReLU_0.4(einsum('hbl,blf->bhf', attn, bank))

Strategy: data-parallel over batch B across 8 NeuronCores (8 b's per core).
Host prep only re-lays-out inputs (transposes / mask bias); all FLOPs on
device.  The dominant k-matmul runs fp32r (full-rate, TF32-style rounding);
the small q / score paths run bf16 (tanh outputs are in [-1,1]).  Scores are
software-pipelined one b-pair behind the k-matmuls so the PE never waits on
the QueryT stream.
"""

import numpy as np
import ml_dtypes

import concourse.bass as bass  # noqa: F401
import concourse.mybir as mybir
import concourse.tile as tile
from concourse import bacc, bass_utils

F32 = mybir.dt.float32
F32R = mybir.dt.float32r
BF16 = mybir.dt.bfloat16
AF = mybir.ActivationFunctionType
AX = mybir.AxisListType

import os as _osd
ALLBF16 = _osd.environ.get("KERNEL_ALLBF16", "0") == "1"
MMDT = BF16 if ALLBF16 else F32R

H, D, E, F = 8, 256, 1536, 768
B, L = 64, 256
NCORES = 8
BPC = B // NCORES          # 8 b's per core
NBP = BPC // 2             # 4 b-pairs per core
EC, FC, DC, LC = E // 128, F // 128, D // 128, L // 128   # 12, 6, 2, 2


def _build_program():
    nc = bacc.Bacc("TRN2", target_bir_lowering=False, debug=False,
                   enable_asserts=False, num_devices=NCORES)
    qt = nc.dram_tensor("qt", [H, E, D], MMDT, kind="ExternalInput").ap()
    kt = nc.dram_tensor("kt", [H, F, D], MMDT, kind="ExternalInput").ap()
    bkt = nc.dram_tensor("bkt", [BPC, F, L], MMDT, kind="ExternalInput").ap()
    bkn = nc.dram_tensor("bkn", [BPC, L, F], MMDT, kind="ExternalInput").ap()
    xt = nc.dram_tensor("xt", [E, BPC], MMDT, kind="ExternalInput").ap()
    mb = nc.dram_tensor("mb", [BPC, H, L], F32, kind="ExternalInput").ap()
    eye = nc.dram_tensor("eye", [16, 16], F32, kind="ExternalInput").ap()
    zq = nc.dram_tensor("zq", [128, NBP * 640], MMDT, kind="ExternalInput").ap()
    out = nc.dram_tensor("out", [BPC, H, F], F32, kind="ExternalOutput").ap()

    with tile.TileContext(nc) as tc:
        with tc.tile_pool(name="const", bufs=1) as cpool, \
             tc.tile_pool(name="weights", bufs=1) as wpool, \
             tc.tile_pool(name="stream", bufs=2) as spool, \
             tc.tile_pool(name="kbuf", bufs=13) as kpool, \
             tc.tile_pool(name="small", bufs=2) as smpool, \
             tc.tile_pool(name="psA", bufs=3, space="PSUM") as psA, \
             tc.tile_pool(name="psB", bufs=2, space="PSUM") as psB, \
             tc.tile_pool(name="psS", bufs=3, space="PSUM") as psS:

            # ---- stream-tile loader (bkt split per fc for fast arrival) --
            def load_bkt(bp):
                bkt_t = spool.tile([128, FC * 512], MMDT, name="bkt_t", tag="bkt_t")
                v = bkt_t[:].rearrange("p (fc b l) -> p fc b l", fc=FC, b=2)
                for fc in range(FC):
                    nc.sync.dma_start(
                        v[:, fc],
                        bkt[2 * bp:2 * bp + 2, fc * 128:(fc + 1) * 128, :]
                        .rearrange("b p l -> p b l"))
                return bkt_t

            def load_bkn_mb(bp):
                bkn_ts = []
                for b2 in range(2):
                    bkn_t = spool.tile([128, LC * F], MMDT,
                                       name=f"bkn_t{b2}", tag=f"bkn_t{b2}")
                    nc.sync.dma_start(
                        bkn_t[:].rearrange("p (lc f) -> p lc f", lc=LC),
                        bkn[2 * bp + b2].rearrange("(lc p) f -> p lc f", p=128))
                    bkn_ts.append(bkn_t)
                mb_ts = []
                for b2 in range(2):
                    mb_t = smpool.tile([8, L], F32, name=f"mb_t{b2}", tag=f"mb_t{b2}")
                    nc.sync.dma_start(mb_t[:], mb[2 * bp + b2])
                    mb_ts.append(mb_t)
                return bkn_ts, mb_ts

            def load_bp_tiles(bp):
                bkt_t = load_bkt(bp)
                bkn_ts, mb_ts = load_bkn_mb(bp)
                return bkt_t, bkn_ts, mb_ts

            # KeyT, all heads, stays resident:  [128, fc*256 + d].
            # kt[0] + bp0's bank tiles are issued first so the PE can start
            # within a couple of microseconds; everything else streams behind.
            kt_tiles = []
            for h in range(H):
                t = wpool.tile([128, FC * D], MMDT, name=f"kt_sb{h}", tag=f"kt_sb{h}")
                kt_tiles.append(t)

            def load_kt(h):
                for piece in range(2):
                    nc.sync.dma_start(
                        kt_tiles[h][:, piece * (FC // 2) * D:
                                    (piece + 1) * (FC // 2) * D]
                        .rearrange("p (fc d) -> p fc d", fc=FC // 2),
                        kt[h, piece * (F // 2):(piece + 1) * (F // 2)]
                        .rearrange("(fc p) d -> p fc d", p=128))

            bkt0_t = spool.tile([128, FC * 512], MMDT, name="bkt_t", tag="bkt_t")
            v0 = bkt0_t[:].rearrange("p (fc b l) -> p fc b l", fc=FC, b=2)

            def load_bkt0_fc(fc):
                nc.sync.dma_start(
                    v0[:, fc],
                    bkt[0:2, fc * 128:(fc + 1) * 128, :].rearrange("b p l -> p b l"))

            import os as _os0
            REPEAT = int(_os0.environ.get("KERNEL_REPEAT", "1"))
            load_bkt0_fc(0)
            for piece in range(3):
                nc.sync.dma_start(
                    kt_tiles[0][:, piece * 2 * D:(piece + 1) * 2 * D]
                    .rearrange("p (fc d) -> p fc d", fc=2),
                    kt[0, piece * 256:(piece + 1) * 256]
                    .rearrange("(fc p) d -> p fc d", p=128))
            for fc in range(1, FC):
                load_bkt0_fc(fc)
            bkt0 = bkt0_t
            for h in range(1, H):
                load_kt(h)
            preloaded = {0: (bkt0, None, None)}

            eye_t = cpool.tile([16, 16], F32)
            xt_sb = cpool.tile([128, EC * BPC], MMDT)
            qz_sb = cpool.tile([128, NBP * 640], MMDT)

            def load_consts():
                nc.sync.dma_start(eye_t[:], eye)
                nc.sync.dma_start(
                    xt_sb[:].rearrange("p (ec b) -> p ec b", ec=EC),
                    xt.rearrange("(ec p) b -> p ec b", p=128))
                nc.sync.dma_start(qz_sb[:], zq)

            def q_phase(heads):
                """q = tanh(x @ Query^T): per h, psum[b=8, d=256] over 12
                E-chunks, then PE-transpose into the zero-padded score lhsT."""
                for h in heads:
                    pq = psS.tile([BPC, D], F32, name="pq", tag="pss")
                    for half in range(2):
                        qt_c = spool.tile([128, EC * D // 2], MMDT,
                                          name="qt_c", tag="qt_c")
                        nc.sync.dma_start(
                            qt_c[:].rearrange("p (ec d) -> p ec d", ec=EC // 2),
                            qt[h, half * (E // 2):(half + 1) * (E // 2)]
                            .rearrange("(ec p) d -> p ec d", p=128))
                        for e2 in range(EC // 2):
                            ec = half * (EC // 2) + e2
                            nc.tensor.matmul(pq[:], xt_sb[:, ec * BPC:(ec + 1) * BPC],
                                             qt_c[:, e2 * D:(e2 + 1) * D],
                                             start=(ec == 0), stop=(ec == EC - 1))
                    q_sb = smpool.tile([BPC, D], F32, name="q_sb", tag="q_sb")
                    nc.scalar.activation(q_sb[:], pq[:], AF.Tanh)
                    for dc in range(DC):
                        pt = psS.tile([128, BPC], F32, name="pt", tag="pss")
                        nc.tensor.transpose(pt[:], q_sb[:, dc * 128:(dc + 1) * 128],
                                            eye_t[0:BPC, 0:BPC])
                        for bp in range(NBP):
                            for b2 in range(2):
                                col = bp * 640 + (2 * h + dc) * 40 + 32 * b2 + h
                                nc.vector.tensor_copy(
                                    qz_sb[:, col:col + 1],
                                    pt[:, bp * 2 + b2:bp * 2 + b2 + 1])

            def compute_k(bp, bkt_t):
                """k = tanh(KeyT^T @ bankT) for all heads of this b-pair."""
                k_tiles = []
                for h in range(H):
                    k_t = kpool.tile([128, DC * 512], MMDT, name="k_t", tag="k_t")
                    for dc in range(DC):
                        pk = psA.tile([128, 512], F32, name="pk", tag="pk")
                        for fc in range(FC):
                            nc.tensor.matmul(
                                pk[:],
                                kt_tiles[h][:, fc * D + dc * 128:
                                            fc * D + dc * 128 + 128],
                                bkt_t[:, fc * 512:(fc + 1) * 512],
                                start=(fc == 0), stop=(fc == FC - 1))
                        nc.scalar.activation(k_t[:, dc * 512:(dc + 1) * 512], pk[:],
                                             AF.Tanh)
                    k_tiles.append(k_t)
                return k_tiles

            def score_phase(bp, k_tiles, bkn_ts, mb_ts, ps40=None):
                # score: accumulate all (h, dc) into one [40, 512] psum
                # (rows b2*32+h; cols 8..31 of each lhsT block are zero)
                if ps40 is None:
                    ps40 = psB.tile([40, 512], F32, name="ps40", tag="ps40")
                    for h in range(H):
                        for dc in range(DC):
                            base = bp * 640 + (2 * h + dc) * 40
                            nc.tensor.matmul(
                                ps40[:],
                                qz_sb[:, base:base + 40],
                                k_tiles[h][:, dc * 512:(dc + 1) * 512],
                                start=(h == 0 and dc == 0),
                                stop=(h == H - 1 and dc == DC - 1))

                # masked softmax over l (free axis); per-b2 tiles at base 0
                pT = smpool.tile([128, 32], MMDT, name="pT", tag="pT")
                rzs = []
                for b2 in range(2):
                    s_sb = smpool.tile([8, L], F32, name=f"s_sb{b2}", tag=f"s_sb{b2}")
                    nc.vector.tensor_add(s_sb[:],
                                         ps40[32 * b2:32 * b2 + 8,
                                              256 * b2:256 * b2 + 256],
                                         mb_ts[b2][:])
                    nmax = smpool.tile([8, 1], F32, name=f"nmax{b2}", tag=f"nmax{b2}")
                    nc.vector.reduce_max(nmax[:], s_sb[:], axis=AX.X, negate=True)
                    p_sb = smpool.tile([8, L], F32, name=f"p_sb{b2}", tag=f"p_sb{b2}")
                    zsum = smpool.tile([8, 1], F32, name=f"zsum{b2}", tag=f"zsum{b2}")
                    nc.scalar.activation(p_sb[:], s_sb[:], AF.Exp, bias=nmax[:],
                                         accum_out=zsum[:])
                    rz = smpool.tile([8, 1], F32, name=f"rz{b2}", tag=f"rz{b2}")
                    nc.vector.reciprocal(rz[:], zsum[:])
                    rzs.append(rz)
                    for lc in range(LC):
                        ptp = psS.tile([128, 8], F32, name="ptp", tag="pss")
                        nc.tensor.transpose(ptp[:], p_sb[:, lc * 128:(lc + 1) * 128],
                                            eye_t[0:8, 0:8])
                        nc.vector.tensor_copy(
                            pT[:, b2 * 16 + lc * 8:b2 * 16 + lc * 8 + 8], ptp[:])

                # emb = attn @ bank, normalize+LeakyReLU fused into Prelu
                import os as _os2
                simsafe = _os2.environ.get("KERNEL_SIM_SAFE", "0") == "1"
                for b2 in range(2):
                    o_sb = smpool.tile([8, F], F32, name=f"o_sb{b2}", tag=f"o_sb{b2}")
                    for fh in range(2):
                        pe = psS.tile([8, 384], F32, name="pe", tag="pss")
                        for lc in range(LC):
                            nc.tensor.matmul(
                                pe[:],
                                pT[:, b2 * 16 + lc * 8:b2 * 16 + lc * 8 + 8],
                                bkn_ts[b2][:, lc * F + fh * 384:
                                            lc * F + fh * 384 + 384],
                                start=(lc == 0), stop=(lc == LC - 1))
                        if simsafe:
                            nc.scalar.activation(o_sb[:, fh * 384:fh * 384 + 384],
                                                 pe[:], AF.Copy, scale=rzs[b2][:])
                        else:
                            nc.scalar.activation(o_sb[:, fh * 384:fh * 384 + 384],
                                                 pe[:], AF.Prelu,
                                                 scale=rzs[b2][:], alpha=0.4)
                    nc.sync.dma_start(out[2 * bp + b2], o_sb[:])

            # ---- main loop: scores pipelined one b-pair behind k ---------
            import os as _os
            PIPELINE = _os.environ.get("KERNEL_NO_PIPE", "0") != "1"
            for rep in range(REPEAT):
              if rep > 0:
                # re-stream everything, same work per repeat
                for h in range(H):
                    load_kt(h)
                preloaded = {0: load_bp_tiles(0)}
              pending = None
              for bp in range(NBP):
                  bkt_t, bkn_ts, mb_ts = preloaded.pop(bp)
                  if bkn_ts is None:
                      bkn_ts, mb_ts = load_bkn_mb(bp)
                  if bp + 1 < NBP:
                      preloaded[bp + 1] = load_bp_tiles(bp + 1)
                  if bp == 0:
                      load_consts()
                  k_tiles = compute_k(bp, bkt_t)
                  if bp == 0:
                      q_phase(range(0, 4))
                  elif bp == 1:
                      q_phase(range(4, 8))
                  if not PIPELINE:
                      score_phase(bp, k_tiles, bkn_ts, mb_ts)
                      continue
                  if pending is not None:
                      score_phase(*pending)
                  pending = (bp, k_tiles, bkn_ts, mb_ts)
              if PIPELINE:
                  score_phase(*pending)

    nc.finalize()
    return nc


def _host_prep(x, bank, mask, Query, Key):
    x = np.ascontiguousarray(x, dtype=np.float32)
    bank = np.ascontiguousarray(bank, dtype=np.float32)
    Query = np.ascontiguousarray(Query, dtype=np.float32)
    Key = np.ascontiguousarray(Key, dtype=np.float32)

    mmdt = ml_dtypes.bfloat16 if ALLBF16 else np.float32
    qt = np.ascontiguousarray(Query.transpose(0, 2, 1)).astype(mmdt)
    kt = np.ascontiguousarray(Key.transpose(0, 2, 1)).astype(mmdt)
    bkt = np.ascontiguousarray(bank.transpose(0, 2, 1)).astype(mmdt)
    bkn = bank.astype(mmdt)
    mbias = np.where(mask == 0, np.float32(-1e8), np.float32(0.0)).astype(np.float32)
    mb = np.ascontiguousarray(np.repeat(mbias[:, None, :], H, axis=1))
    eye = np.eye(16, dtype=np.float32)
    zq = np.zeros((128, NBP * 640), dtype=mmdt)

    in_maps = []
    for c in range(NCORES):
        bs = c * BPC
        in_maps.append({
            "qt": qt,
            "kt": kt,
            "bkt": np.ascontiguousarray(bkt[bs:bs + BPC]),
            "bkn": np.ascontiguousarray(bkn[bs:bs + BPC]),
            "xt": np.ascontiguousarray(x[bs:bs + BPC].T).astype(mmdt),
            "mb": np.ascontiguousarray(mb[bs:bs + BPC]),
            "eye": eye,
            "zq": zq,
        })
    return in_maps


_NC_CACHE = {}


def kernel(x, bank, mask, Query, Key):
    import os
    if "nc" not in _NC_CACHE:
        _NC_CACHE["nc"] = _build_program()
    nc = _NC_CACHE["nc"]
    in_maps = _host_prep(x, bank, mask, Query, Key)

    trace = os.environ.get("KERNEL_TRACE", "0") == "1"
    res = bass_utils.run_bass_kernel_spmd(nc, in_maps,
                                          core_ids=list(range(NCORES)),
                                          trace=trace)
    if trace:
        print("exec_time_ns:", res.exec_time_ns,
              "mean:", res.mean_exec_time_ns,
              "core:", res.max_exec_time_core_id)
    return np.concatenate([r["out"] for r in res.results], axis=0)



# revision 10
# speedup vs baseline: 1.4764x; 1.4764x over previous
"""Trainium2 Bass kernel for nn_AttentionModule (sparse_attention).

Computation (reference):
  q = tanh(einsum('hde,be->hbd', Query, x))          H=8 D=256 E=1536
  k = tanh(einsum('hdf,blf->hbld', Key, bank))       B=64 L=256 F=768
  s = einsum('hbld,hbd->hbl', k, q)  masked softmax over l
  out = LeakyReLU_0.4(einsum('hbl,blf->bhf', attn, bank))

Strategy: data-parallel over batch B across 8 NeuronCores (8 b's per core).

Sparsity: the mask zeroes ~half the L positions; masked positions receive
-1e8 bias so their softmax weight is ~0 and they contribute nothing to the
output.  Host prep therefore COMPACTS bank per-b to the unmasked columns
(padded to Lp, a multiple of 32; padded slots keep the -1e8 bias), which
cuts the dominant k-matmul, the score matmul and the softmax by L/Lp
(~1.6x).  Compaction is a gather (re-layout); all FLOPs stay on device.

Device pipeline per core (PE stream is issued to stay gap-free):
  - k = tanh(KeyT^T @ bankT) head-outer (KeyT streams one head per ~5us of
    PE work), all 4 b-pairs; moving dim 2*Lp>=256 so fp32r runs full rate.
  - q-heads (bf16, halves the Query DMA) interleaved into the first k-heads
    to cover the DMA lead-in; q is DVE-block-transposed into the zero-padded
    score lhsT (qz) -- no PE transposes anywhere.
  - scores: all (h,dc) accumulate into one [40, 2*Lp] psum per b-pair;
    masked softmax on ACT/DVE; attn DVE-block-transposed; emb = attn @ bank
    with normalize+LeakyReLU fused into one Prelu activation.
"""

import os
import numpy as np
import concourse.bass as bass  # noqa: F401
import concourse.mybir as mybir
import concourse.tile as tile
from concourse import bacc, bass_utils

F32 = mybir.dt.float32
F32R = mybir.dt.float32r
FP16 = mybir.dt.float16
AF = mybir.ActivationFunctionType
AX = mybir.AxisListType

# dtype of the big k-matmul operands (KeyT / bankT). fp16 halves their DMA
# at ~0.05% quantization error; fp32r keeps tf32-grade accuracy.
KF16 = os.environ.get("KERNEL_KF16", "1") == "1"

H, D, E, F = 8, 256, 1536, 768
B, L = 64, 256
NCORES = 8
BPC = B // NCORES          # 8 b's per core
NBP = BPC // 2             # 4 b-pairs per core
EC, FC, DC = E // 128, F // 128, D // 128   # 12, 6, 2


def _build_program(Lp, kf16):
    KMM = FP16 if kf16 else F32R
    N2 = 2 * Lp                 # k / score moving width per b-pair
    NBLK = Lp // 32             # DVE 32x32 transpose blocks per attn stack
    L_REM = Lp - 128 if Lp > 128 else 0   # l rows beyond the first 128

    nc = bacc.Bacc("TRN2", target_bir_lowering=False, debug=False,
                   enable_asserts=False, num_devices=NCORES)
    qt = nc.dram_tensor("qt", [H, E, D], FP16, kind="ExternalInput").ap()
    kt = nc.dram_tensor("kt", [H, F, D], KMM, kind="ExternalInput").ap()
    bkt = nc.dram_tensor("bkt", [NBP, F, N2], KMM, kind="ExternalInput").ap()
    bkn = nc.dram_tensor("bkn", [BPC, Lp, F], FP16, kind="ExternalInput").ap()
    xt = nc.dram_tensor("xt", [128, EC * BPC], FP16, kind="ExternalInput").ap()
    mb = nc.dram_tensor("mb", [BPC, H, Lp], F32, kind="ExternalInput").ap()
    out = nc.dram_tensor("out", [BPC, H, F], F32, kind="ExternalOutput").ap()

    with tile.TileContext(nc) as tc:
        with tc.tile_pool(name="const", bufs=1) as cpool, \
             tc.tile_pool(name="weights", bufs=1) as wpool, \
             tc.tile_pool(name="stream", bufs=2) as spool, \
             tc.tile_pool(name="small", bufs=2) as smpool, \
             tc.tile_pool(name="psK", bufs=3, space="PSUM") as psK, \
             tc.tile_pool(name="psS", bufs=2, space="PSUM") as psS, \
             tc.tile_pool(name="psM", bufs=3, space="PSUM") as psM:

            # ---------------- resident SBUF tiles ------------------------
            kt_tiles = [wpool.tile([128, FC * D], KMM, name=f"kt_sb{h}",
                                   tag=f"kt_sb{h}") for h in range(H)]
            # bankT, all b-pairs resident: [128(f), bp, fc, (b2 l)]
            bktA = cpool.tile([128, NBP * FC * N2], KMM)
            bktA_v = bktA[:].rearrange("p (bp fc n) -> p bp fc n", bp=NBP, fc=FC)
            # bank (natural layout), emb rhs: first 128 l-rows + remainder.
            bkn0 = cpool.tile([128, BPC * F], FP16, name="bkn0")
            bkn1 = (cpool.tile([L_REM, BPC * F], FP16, name="bkn1")
                    if L_REM else None)
            xt_sb = cpool.tile([128, EC * BPC], FP16)
            mbA = cpool.tile([8, BPC * Lp], F32)
            # zero-padded score lhsT: col = bp*640 + (2h+dc)*40 + 32*b2 + h
            qz = cpool.tile([128, NBP * 640], FP16)
            qz_v = qz[:].rearrange("p (bp blk c) -> p bp blk c", bp=NBP, blk=16)
            # k = tanh(...), all heads/pairs resident: [128(d), h, dc, bp, n2]
            k_t = cpool.tile([128, H * DC * NBP * N2], FP16)
            k_v = k_t[:].rearrange("p (h dc bp n) -> p h dc bp n",
                                   h=H, dc=DC, bp=NBP)

            # ---------------- DMA issue helpers --------------------------
            def load_xt_mb():
                nc.sync.dma_start(xt_sb[:], xt)
                nc.sync.dma_start(
                    mbA[:].rearrange("h (b l) -> h b l", b=BPC),
                    mb.rearrange("b h l -> h b l"))

            def load_kt(h, pieces=2):
                fc_per = FC // pieces
                for piece in range(pieces):
                    nc.sync.dma_start(
                        kt_tiles[h][:, piece * fc_per * D:(piece + 1) * fc_per * D]
                        .rearrange("p (fc d) -> p fc d", fc=fc_per),
                        kt[h, piece * fc_per * 128:(piece + 1) * fc_per * 128]
                        .rearrange("(fc p) d -> p fc d", p=128))

            def load_bktA(bp, pieces=2):
                fc_per = FC // pieces
                for piece in range(pieces):
                    nc.sync.dma_start(
                        bktA_v[:, bp, piece * fc_per:(piece + 1) * fc_per],
                        bkt[bp, piece * fc_per * 128:(piece + 1) * fc_per * 128]
                        .rearrange("(fc p) n -> p fc n", p=128))

            def load_bkn():
                for b in range(BPC):
                    nc.sync.dma_start(
                        bkn0[:, b * F:(b + 1) * F], bkn[b, 0:128])
                    if L_REM:
                        nc.sync.dma_start(
                            bkn1[:, b * F:(b + 1) * F], bkn[b, 128:128 + L_REM])

            # ---------------- q phase (bf16) ------------------------------
            # q = tanh(x @ Query^T); two stacks of 4 heads (rows 32*hh, 8
            # live rows each -- engine writes need 32-aligned partition
            # bases) -> DVE 32x32 block transposes -> strided copies into qz.
            q_stacks = [smpool.tile([128, D], FP16, name=f"qs{g}", tag=f"qs{g}")
                        for g in range(2)]

            def q_head(h):
                g, hh = divmod(h, 4)
                pq = psM.tile([BPC, D], F32, name="pq", tag="psm")
                for half in range(2):
                    qt_c = spool.tile([128, (EC // 2) * D], FP16,
                                      name="qt_c", tag="qt_c")
                    nc.sync.dma_start(
                        qt_c[:].rearrange("p (ec d) -> p ec d", ec=EC // 2),
                        qt[h, half * (E // 2):(half + 1) * (E // 2)]
                        .rearrange("(ec p) d -> p ec d", p=128))
                    for e2 in range(EC // 2):
                        ec = half * (EC // 2) + e2
                        nc.tensor.matmul(pq[:], xt_sb[:, ec * BPC:(ec + 1) * BPC],
                                         qt_c[:, e2 * D:(e2 + 1) * D],
                                         start=(ec == 0), stop=(ec == EC - 1))
                nc.scalar.activation(q_stacks[g][32 * hh:32 * hh + 8, :], pq[:],
                                     AF.Tanh)

            def q_scatter(g):
                for dc in range(DC):
                    qT = smpool.tile([128, 128], FP16, name=f"qT{g}{dc}",
                                     tag="qT")
                    for i in range(4):          # head row-blocks
                        for j in range(4):      # d sub-blocks
                            nc.vector.transpose(
                                qT[32 * j:32 * j + 32, 32 * i:32 * i + 32],
                                q_stacks[g][32 * i:32 * i + 32,
                                            dc * 128 + 32 * j:dc * 128 + 32 * j + 32])
                    # qT col = 32*hh + 2*bp + b2 (b = 2bp+b2 local batch)
                    qT_v = qT[:].rearrange("p (hh bpx b2) -> p hh bpx b2",
                                           hh=4, bpx=16)
                    for hh in range(4):
                        h = 4 * g + hh
                        for b2 in range(2):
                            nc.vector.tensor_copy(
                                qz_v[:, :, 2 * h + dc, 32 * b2 + h],
                                qT_v[:, hh, 0:4, b2])

            # ---------------- k phase -------------------------------------
            def k_head(h, bps=range(NBP)):
                for bp in bps:
                    for dc in range(DC):
                        pk = psK.tile([128, N2], F32, name="pk", tag="pk")
                        for fc in range(FC):
                            nc.tensor.matmul(
                                pk[:],
                                kt_tiles[h][:, fc * D + dc * 128:
                                            fc * D + dc * 128 + 128],
                                bktA_v[:, bp, fc],
                                start=(fc == 0), stop=(fc == FC - 1))
                        nc.scalar.activation(k_v[:, h, dc, bp], pk[:], AF.Tanh)

            # ---------------- score / softmax / emb -----------------------
            simsafe = os.environ.get("KERNEL_SIM_SAFE", "0") == "1"

            def score_mms(bp):
                ps40 = psS.tile([40, N2], F32, name="ps40", tag="ps40")
                for h in range(H):
                    for dc in range(DC):
                        nc.tensor.matmul(
                            ps40[:], qz_v[:, bp, 2 * h + dc],
                            k_v[:, h, dc, bp],
                            start=(h == 0 and dc == 0),
                            stop=(h == H - 1 and dc == DC - 1))
                return ps40

            def softmax(bp, ps40):
                # attn stack: b2 at rows 0 / 32 (32-aligned engine writes);
                # transposed tiles have head-columns at 32*b2.
                p32 = smpool.tile([64, Lp], FP16, name="p32", tag="p32")
                rzs = []
                for b2 in range(2):
                    b = 2 * bp + b2
                    s_sb = smpool.tile([8, Lp], F32, name=f"s{b2}", tag=f"s{b2}")
                    nc.vector.tensor_add(s_sb[:],
                                         ps40[32 * b2:32 * b2 + 8,
                                              Lp * b2:Lp * b2 + Lp],
                                         mbA[:, b * Lp:(b + 1) * Lp])
                    nmax = smpool.tile([8, 1], F32, name=f"nm{b2}", tag=f"nm{b2}")
                    nc.vector.reduce_max(nmax[:], s_sb[:], axis=AX.X, negate=True)
                    zsum = smpool.tile([8, 1], F32, name=f"zs{b2}", tag=f"zs{b2}")
                    nc.scalar.activation(p32[32 * b2:32 * b2 + 8, :], s_sb[:],
                                         AF.Exp, bias=nmax[:], accum_out=zsum[:])
                    rz = smpool.tile([8, 1], F32, name=f"rz{b2}", tag=f"rz{b2}")
                    nc.vector.reciprocal(rz[:], zsum[:])
                    rzs.append(rz)
                pT0 = smpool.tile([128, 64], FP16, name="pT0", tag="pT0")
                pT1 = (smpool.tile([max(L_REM, 32), 64], FP16,
                                   name="pT1", tag="pT1")
                       if L_REM else None)
                for i in range(2):              # b2 row-blocks
                    for j in range(NBLK):
                        row = 32 * j
                        dst = (pT0[row:row + 32, 32 * i:32 * i + 32]
                               if row < 128
                               else pT1[row - 128:row - 96, 32 * i:32 * i + 32])
                        nc.vector.transpose(
                            dst, p32[32 * i:32 * i + 32, row:row + 32])
                return rzs, pT0, pT1

            def emb(bp, rzs, pT0, pT1):
                for b2 in range(2):
                    b = 2 * bp + b2
                    o_sb = smpool.tile([8, F], F32, name=f"o{b2}", tag=f"o{b2}")
                    for fh in range(2):
                        pe = psM.tile([8, 384], F32, name="pe", tag="psm")
                        nc.tensor.matmul(
                            pe[:], pT0[:, 32 * b2:32 * b2 + 8],
                            bkn0[:, b * F + fh * 384:b * F + fh * 384 + 384],
                            start=True, stop=(L_REM == 0))
                        if L_REM:
                            nc.tensor.matmul(
                                pe[:], pT1[0:L_REM, 32 * b2:32 * b2 + 8],
                                bkn1[:, b * F + fh * 384:b * F + fh * 384 + 384],
                                start=False, stop=True)
                        if simsafe:
                            nc.scalar.activation(o_sb[:, fh * 384:fh * 384 + 384],
                                                 pe[:], AF.Copy, scale=rzs[b2][:])
                        else:
                            nc.scalar.activation(o_sb[:, fh * 384:fh * 384 + 384],
                                                 pe[:], AF.Prelu,
                                                 scale=rzs[b2][:], alpha=0.4)
                    nc.sync.dma_start(out[b], o_sb[:])

            # ---------------- program order -------------------------------
            # DMA queue and PE stream are co-scheduled: q-heads fill the PE
            # while KeyT/bankT stream in; kt[h] always one head ahead.
            nc.vector.memset(qz[:], 0.0)
            load_xt_mb()
            q_head(0)               # qt[0] DMA + q0 MMs
            load_kt(0, pieces=3)
            q_head(1)
            load_bktA(0)
            k_head(0, [0])
            load_bktA(1)
            k_head(0, [1])
            q_head(2)
            load_bktA(2)
            k_head(0, [2])
            q_head(3)
            load_bktA(3)
            k_head(0, [3])
            load_kt(1)
            q_scatter(0)
            k_head(1)
            load_kt(2)
            q_head(4)
            q_head(5)
            k_head(2)
            load_kt(3)
            q_head(6)
            q_head(7)
            k_head(3)
            load_kt(4)
            q_scatter(1)
            k_head(4)
            load_kt(5)
            k_head(5)
            load_kt(6)
            k_head(6)
            load_kt(7)
            load_bkn()
            k_head(7)

            # scores pipelined ahead of softmax/emb chains
            ps0 = score_mms(0)
            sm0 = softmax(0, ps0)
            ps1 = score_mms(1)
            sm1 = softmax(1, ps1)
            emb(0, *sm0)
            ps2 = score_mms(2)
            sm2 = softmax(2, ps2)
            emb(1, *sm1)
            ps3 = score_mms(3)
            sm3 = softmax(3, ps3)
            emb(2, *sm2)
            emb(3, *sm3)

    nc.finalize()
    return nc


def _host_prep(x, bank, mask, Query, Key, Lp, kf16):
    x = np.ascontiguousarray(x, dtype=np.float32)
    bank = np.ascontiguousarray(bank, dtype=np.float32)
    Query = np.ascontiguousarray(Query, dtype=np.float32)
    Key = np.ascontiguousarray(Key, dtype=np.float32)
    mask = np.asarray(mask)

    kdt = np.float16 if kf16 else np.float32
    qt = np.ascontiguousarray(Query.transpose(0, 2, 1)).astype(np.float16)
    kt = np.ascontiguousarray(Key.transpose(0, 2, 1)).astype(kdt)  # [H, F, D]

    # per-b compaction of bank to its unmasked columns, padded to Lp
    if Lp == L:
        bank_c = bank
        mbias = np.where(mask == 0, np.float32(-1e8), np.float32(0.0))
    else:
        idx = np.zeros((B, Lp), dtype=np.int64)
        mbias = np.full((B, Lp), np.float32(-1e8))
        for b in range(B):
            nz = np.flatnonzero(mask[b])
            idx[b, :len(nz)] = nz
            mbias[b, :len(nz)] = 0.0
        bank_c = np.take_along_axis(bank, idx[:, :, None], axis=1)
    mbias = mbias.astype(np.float32)

    in_maps = []
    for c in range(NCORES):
        bs = c * BPC
        bc = bank_c[bs:bs + BPC]                      # [BPC, Lp, F]
        # bkt: [NBP, F, 2*Lp] -- b-pair minor so one DMA pair per bp
        bkt_c = np.ascontiguousarray(
            bc.reshape(NBP, 2, Lp, F).transpose(0, 3, 1, 2)
            .reshape(NBP, F, 2 * Lp)).astype(kdt)
        xs = x[bs:bs + BPC]                           # [BPC, E]
        xt_c = np.ascontiguousarray(
            xs.T.reshape(EC, 128, BPC).transpose(1, 0, 2)
            .reshape(128, EC * BPC)).astype(np.float16)
        mb_c = np.ascontiguousarray(
            np.repeat(mbias[bs:bs + BPC, None, :], H, axis=1))
        in_maps.append({
            "qt": qt,
            "kt": kt,
            "bkt": bkt_c,
            "bkn": np.ascontiguousarray(bc).astype(np.float16),
            "xt": xt_c,
            "mb": mb_c,
        })
    return in_maps


_NC_CACHE = {}


def _pick_lp(mask):
    counts = np.asarray(mask).astype(bool).sum(axis=1)
    if counts.min() == 0:
        return L
    return int(min(L, max(128, -(-int(counts.max()) // 32) * 32)))


def kernel(x, bank, mask, Query, Key):
    Lp = _pick_lp(mask)
    key = (Lp, KF16)
    if key not in _NC_CACHE:
        _NC_CACHE[key] = _build_program(Lp, KF16)
    nc = _NC_CACHE[key]
    in_maps = _host_prep(x, bank, mask, Query, Key, Lp, KF16)

    trace = os.environ.get("KERNEL_TRACE", "0") == "1"
    res = bass_utils.run_bass_kernel_spmd(nc, in_maps,
                                          core_ids=list(range(NCORES)),
                                          trace=trace)
    if trace:
        print("exec_time_ns:", res.exec_time_ns,
              "mean:", res.mean_exec_time_ns,
              "core:", res.max_exec_time_core_id)
    return np.concatenate([r["out"] for r in res.results], axis=0)
